# revision 33
# baseline (speedup 1.0000x reference)
"""ChessNNUE Trainium2 kernel.

Fast path (sparse embedding-lookup; used when the feature matrices and stm
are binary 0/1, which they are for HalfKP-style NNUE inputs):
  - The dense FT matmul (85.9 GFLOP/core -> 1.09 ms PE-bound) is replaced
    by hardware gather: each batch row has ~30 active features, so
      acc[b, :] = ft_b + sum_{k active} ft_w.T[k, :]
  - Host extracts active indices, bin-packs the 4096 rows into 32
    (core, batch-tile) bins of 128 rows (greedy on totals + swap
    hill-climb on the max per-stream length, which sets ga/gb), and
    builds int16 gather-index streams, each sorted by table index for
    DRAM page locality (split into a <32767 "low" table and a "high"
    table because dma_gather indices are int16).
  - The table is fp8 e4m3 scaled by TSCALE (halves gather traffic vs
    bf16; end-to-end rel err ~3e-3 vs the 2e-2 gate).
  - On device, nc.gpsimd.dma_gather pulls active table rows into SBUF
    ([128 slots, G, 1024] fp8); 0/1 segment matrices S (fp8, built once
    per dispatch on DVE via iota==segid; pad slots carry segid=-1 so
    they hit no row) reduce slots into per-row accumulators on the
    tensor engine with fp8 DoubleRow matmuls (two K-tiles per pass):
    PSUM[b, hid] = S.T @ gathered. ft_b (scaled) is folded in via an
    identity-matmul K-tile over a broadcast bias tile.
  - Engine assignment keeps the gather pipeline free: Pool does ONLY
    gather desc-gen; DVE clips the accumulators straight out of PSUM
    (h1, scaled by TSCALE with clip bounds 0..TSCALE; 1/TSCALE is folded
    into l1_w on the host); PE transposes h1 back to [hid, batch].
  - stm is binary, so clip commutes with the wb/bw select, and the
    select is pushed through the linear l1 layer: A = l1w@[w;b],
    B = l1w@[b;w], h2pre = B + stm*(A-B) on the tiny [64, BC] tile.
  - HBM traffic/core drops from 335 MB (dense f32) to ~32 MB.

Measured on trn2: ~120 us per rep for all 8 cores (baseline dense
kernel: 380 us), raw rel err ~3.3e-3.

Fallback path: the original dense data-parallel kernel (features bf16
matmul over all 40960 features), used if inputs are not binary.
"""

import math
import numpy as np
from contextlib import ExitStack

import concourse.bass as bass
import concourse.tile as tile
from concourse import bacc, mybir
from concourse.bass_utils import run_bass_kernel_spmd

B, FEAT, HID = 4096, 40960, 1024
L1, L2 = 64, 32
NCORES = 8
BC = B // NCORES          # 512 batch rows per core
BT = BC // 128            # 4 batch tiles of 128 rows per core
NHC = HID // 128          # 8 hid chunks of 128

SPLIT = 30720             # features < SPLIT go to the low table (int16
                          # gather indices cap it at 32767; 30720 makes the
                          # lo streams fit 23 K-tiles = 3 full calls and the
                          # hi streams 8 K-tiles = 1 call, the 31-tile
                          # optimum for ~30 active features/row)
NTA = 32768               # low-table row span (tblB base offset)
NTB = FEAT - SPLIT + 1    # high table rows
GCAP = 38                 # max total K-tiles per stream (SBUF budget:
                          # ~4.15KB/partition per K-tile at gbufs=3)
GCALL = 8                 # K-tiles per dma_gather call (SWDGE ring holds
                          # 1024 descriptors; 2048-desc calls desync the hw)
GBUFS = 3                 # gather tile pool depth

F32 = mybir.dt.float32
BF16 = mybir.dt.bfloat16
F8 = mybir.dt.float8e4
I16 = mybir.dt.int16
BF16_NP = mybir.dt.np(BF16)
F8_NP = mybir.dt.np(F8)

# The ft table is stored in fp8 e4m3 scaled by TSCALE so the ~N(0, 0.02)
# weights land in e4m3's normal range (rel err ~2^-4 per element; final
# output rel err ~2.3e-3, well under the 2e-2 gate). The accumulated
# activations stay scaled by TSCALE until the h1 clip (bounds 0..TSCALE);
# 1/TSCALE is folded into l1_w on the host.
TSCALE = 64.0

_CACHE = {}


# --------------------------------------------------------------------------
# Sparse (embedding lookup) path
# --------------------------------------------------------------------------

def _build_sparse(ga, gb, reps=1, mode="full", nq=4, dr=True, gcall=GCALL,
                  scratch=16384, gbufs=GBUFS):
    """Per-core Bass program for the gather path.

    ga/gb: K-tiles (128 gather slots each) per (batch-tile, perspective)
    stream for the low/high table halves.
    mode: "full" | "dmaonly" (gathers only) — ablation builds.
    dr: use fp8 DoubleRow paired segment matmuls (else bf16 S, singles).

    Engine assignment (the pipeline lives or dies on this):
      Pool   - ONLY gather desc-gen, so stream i+1's DMA is never
               head-of-line blocked behind stream i's compute.
      PE     - bias + segment matmuls, h1 transposes, head matmuls.
      DVE    - S-builds (once per dispatch; seg streams are rep-constant),
               PSUM clip to h1, transpose copies, the post-l1 stm select.
    stm is binary here, so clip commutes with the wb/bw select and the
    select itself is algebraically pushed through the (linear) l1 layer:
    A = l1w@[w;b], B = l1w@[b;w], h2pre = B + stm*(A-B) on [L1, BC].
    """
    Alu = mybir.AluOpType
    Act = mybir.ActivationFunctionType
    do_dma = mode in ("full", "dmaonly")
    do_pe = mode in ("full",)

    nc = bacc.Bacc("TRN2", target_bir_lowering=False, debug=False,
                   num_devices=NCORES, num_swdge_queues=nq,
                   dynamic_dma_scratch_size=scratch)
    qc = [0]

    tbl = nc.dram_tensor("tbl", (NTA + NTB, HID), F8, kind="ExternalInput")
    idxlo = nc.dram_tensor("idxlo", (128, BT * 2 * ga * 8), I16,
                           kind="ExternalInput")
    seglo = nc.dram_tensor("seglo", (128, BT * 2 * ga), F32,
                           kind="ExternalInput")
    seg2lo = nc.dram_tensor("seg2lo", (128, BT * 2 * ga), F32,
                            kind="ExternalInput")
    if gb:
        idxhi = nc.dram_tensor("idxhi", (128, BT * 2 * gb * 8), I16,
                               kind="ExternalInput")
        seghi = nc.dram_tensor("seghi", (128, BT * 2 * gb), F32,
                               kind="ExternalInput")
        seg2hi = nc.dram_tensor("seg2hi", (128, BT * 2 * gb), F32,
                                kind="ExternalInput")
    stmv = nc.dram_tensor("stmv", (1, BC), BF16, kind="ExternalInput")
    iota = nc.dram_tensor("iota", (128, 128), F32, kind="ExternalInput")
    ident = nc.dram_tensor("ident", (128, 128), BF16, kind="ExternalInput")
    biasb = nc.dram_tensor("biasb", (128, HID), BF16, kind="ExternalInput")
    l1w = nc.dram_tensor("l1w", (2 * HID, L1), BF16, kind="ExternalInput")
    l1b = nc.dram_tensor("l1b", (L1,), F32, kind="ExternalInput")
    l2w = nc.dram_tensor("l2w", (L1, L2), BF16, kind="ExternalInput")
    l2b = nc.dram_tensor("l2b", (L2,), F32, kind="ExternalInput")
    l3w = nc.dram_tensor("l3w", (L2, 1), BF16, kind="ExternalInput")
    l3b = nc.dram_tensor("l3b", (1,), F32, kind="ExternalInput")
    out = nc.dram_tensor("out", (2, BC), F32, kind="ExternalOutput")

    tblA = tbl.ap()[0:NTA, :]
    tblB = tbl.ap()[NTA:NTA + NTB, :]
    sdt = F8 if dr else BF16

    with ExitStack() as ctx:
        tc = ctx.enter_context(tile.TileContext(nc))
        const = ctx.enter_context(tc.tile_pool(name="const", bufs=1))
        glopool = ctx.enter_context(tc.tile_pool(name="glopool", bufs=gbufs))
        ghipool = ctx.enter_context(tc.tile_pool(name="ghipool", bufs=gbufs))
        spool = ctx.enter_context(tc.tile_pool(name="spool", bufs=1))
        tmppool = ctx.enter_context(tc.tile_pool(name="tmppool", bufs=1))
        h1pool = ctx.enter_context(tc.tile_pool(name="h1pool", bufs=2))
        h1Tpool = ctx.enter_context(tc.tile_pool(name="h1Tpool", bufs=1))
        psum = ctx.enter_context(
            tc.tile_pool(name="psum", bufs=6, space="PSUM"))

        # ---------- constants ----------
        iota_sb = const.tile([128, 128], F32)
        nc.sync.dma_start(iota_sb[:], iota.ap())
        ident_sb = const.tile([128, 128], BF16)
        nc.sync.dma_start(ident_sb[:], ident.ap())
        biasb_sb = const.tile([128, HID], BF16)
        nc.sync.dma_start(biasb_sb[:], biasb.ap())
        stmv_sb = const.tile([1, BC], BF16)
        nc.sync.dma_start(stmv_sb[:], stmv.ap())
        idxlo_sb = const.tile([128, BT * 2 * ga * 8], I16)
        nc.sync.dma_start(idxlo_sb[:], idxlo.ap())
        seglo_sb = const.tile([128, BT * 2 * ga], F32)
        nc.sync.dma_start(seglo_sb[:], seglo.ap())
        seg2lo_sb = const.tile([128, BT * 2 * ga], F32)
        nc.sync.dma_start(seg2lo_sb[:], seg2lo.ap())
        if gb:
            idxhi_sb = const.tile([128, BT * 2 * gb * 8], I16)
            nc.sync.dma_start(idxhi_sb[:], idxhi.ap())
            seghi_sb = const.tile([128, BT * 2 * gb], F32)
            nc.sync.dma_start(seghi_sb[:], seghi.ap())
            seg2hi_sb = const.tile([128, BT * 2 * gb], F32)
            nc.sync.dma_start(seg2hi_sb[:], seg2hi.ap())

        l1b_sb = const.tile([L1, 1], F32)
        nc.sync.dma_start(l1b_sb[:], l1b.ap())
        l2b_sb = const.tile([L2, 1], F32)
        nc.sync.dma_start(l2b_sb[:], l2b.ap())
        l3b_sb = const.tile([1, 1], F32)
        nc.sync.dma_start(l3b_sb[:], l3b.ap())
        l1w_sb = const.tile([128, (2 * HID) // 128, L1], BF16)
        nc.sync.dma_start(l1w_sb[:],
                          l1w.ap().rearrange("(t p) m -> p t m", p=128))
        l2w_sb = const.tile([L1, L2], BF16)
        nc.sync.dma_start(l2w_sb[:], l2w.ap())
        l3w_sb = const.tile([L2, 1], BF16)
        nc.sync.dma_start(l3w_sb[:], l3w.ap())

        # stm broadcast across L1 partitions: [L1, BC] = ones[1,L1].T @ stm
        ones_bf = const.tile([1, L1], BF16)
        nc.vector.memset(ones_bf[:], 1.0)
        ps_stm = psum.tile([L1, BC], F32, tag="ps")
        nc.tensor.matmul(ps_stm[:], ones_bf[:], stmv_sb[:],
                         start=True, stop=True)
        stmb_sb = const.tile([L1, BC], F32)
        nc.vector.tensor_copy(stmb_sb[:], ps_stm[:])

        # h1Ts[c] = transposed clipped white acc chunk c (c=0..7),
        # h1Ts[8+c] = black
        h1Ts = [h1Tpool.tile([128, BC], BF16, tag=f"h1T{t}", name=f"h1T{t}")
                for t in range(16)]

        # ---- S matrices for all 8 streams: seg streams are constant
        # across reps, so these are built once per dispatch. Each slot can
        # target up to two rows (in-stream duplicate merge):
        # S = (iota==seg1) + (iota==seg2); rows are distinct so S stays 0/1.
        def build_s(tag, g, seg_sb, seg2_sb, sidx):
            S = spool.tile([128, g, 128], sdt, tag=f"{tag}{sidx}")
            iota_bc = (iota_sb[:].rearrange("p b -> p () b")
                       .broadcast_to([128, g, 128]))
            nc.vector.tensor_tensor(
                S[:], iota_bc,
                seg_sb[:, sidx * g:(sidx + 1) * g]
                .rearrange("p g -> p g ()").broadcast_to([128, g, 128]),
                Alu.is_equal)
            S2 = spool.tile([128, g, 128], sdt, tag=f"{tag}2tmp")
            nc.vector.tensor_tensor(
                S2[:], iota_bc,
                seg2_sb[:, sidx * g:(sidx + 1) * g]
                .rearrange("p g -> p g ()").broadcast_to([128, g, 128]),
                Alu.is_equal)
            nc.vector.tensor_add(S[:], S[:], S2[:])
            return S

        Slos, Shis = [], []
        for sidx in range(BT * 2):
            Slos.append(build_s("Slo", ga, seglo_sb, seg2lo_sb, sidx))
            if gb:
                Shis.append(build_s("Shi", gb, seghi_sb, seg2hi_sb, sidx))

        def emit_body():
            for bt in range(BT):
                psb = {}
                for p in range(2):
                    sidx = bt * 2 + p
                    # SWDGE queue ring caps one dma_gather at 1024 descs
                    # -> 8 K-tiles per call.
                    glo_t = glopool.tile([128, ga, HID], F8, tag="glo",
                                         name="glo")
                    ghi_t = (ghipool.tile([128, gb, HID], F8, tag="ghi",
                                          name="ghi") if gb else None)
                    if do_dma:
                        for g0 in range(0, ga, gcall):
                            g1 = min(g0 + gcall, ga)
                            nc.gpsimd.dma_gather(
                                glo_t[:, g0:g1, :], tblA,
                                idxlo_sb[:, sidx * ga * 8 + g0 * 8:
                                         sidx * ga * 8 + g1 * 8],
                                (g1 - g0) * 128, (g1 - g0) * 128, HID,
                                queue_num=qc[0] % nq)
                            qc[0] += 1
                        for g0 in range(0, gb, gcall):
                            g1 = min(g0 + gcall, gb)
                            nc.gpsimd.dma_gather(
                                ghi_t[:, g0:g1, :], tblB,
                                idxhi_sb[:, sidx * gb * 8 + g0 * 8:
                                         sidx * gb * 8 + g1 * 8],
                                (g1 - g0) * 128, (g1 - g0) * 128, HID,
                                queue_num=qc[0] % nq)
                            qc[0] += 1

                    if not do_pe:
                        continue
                    Slo, Shi = Slos[sidx], (Shis[sidx] if gb else None)
                    ps0 = psum.tile([128, 512], F32, tag="ps")
                    ps1 = psum.tile([128, 512], F32, tag="ps")
                    halves = [(Slo, glo_t, ga)]
                    if gb:
                        halves.append((Shi, ghi_t, gb))
                    # long same-bank matmul runs keep PE streaming; fp8
                    # DoubleRow packs two K-tiles per pass
                    for h, ps in enumerate((ps0, ps1)):
                        hs = slice(h * 512, (h + 1) * 512)
                        nc.tensor.matmul(ps[:], ident_sb[:], biasb_sb[:, hs],
                                         start=True, stop=False)
                        for hi, (S, gt, gn) in enumerate(halves):
                            is_last = hi == len(halves) - 1
                            if not dr:
                                for g in range(gn):
                                    nc.tensor.matmul(
                                        ps[:], S[:, g, :], gt[:, g, hs],
                                        start=False,
                                        stop=is_last and g == gn - 1)
                                continue
                            for g in range(0, gn - 1, 2):
                                nc.tensor.matmul(
                                    ps[:], S[:, g:g + 2, :],
                                    gt[:, g:g + 2, hs],
                                    start=False,
                                    stop=is_last and g + 2 >= gn,
                                    perf_mode=mybir.MatmulPerfMode.DoubleRow)
                            if gn % 2:
                                nc.tensor.matmul(ps[:], S[:, gn - 1, :],
                                                 gt[:, gn - 1, hs],
                                                 start=False, stop=is_last)
                    psb[p] = (ps0, ps1)

                # clip straight out of PSUM (stm select happens after l1)
                for p in range(2 if do_pe else 0):
                    for h in range(2):
                        ps = psb[p][h]
                        h1 = h1pool.tile([128, 512], BF16, tag=f"h1_{p}{h}")
                        nc.vector.tensor_scalar(h1[:], ps[:], 0.0, TSCALE,
                                                Alu.max, Alu.min)
                        for q in range(4):
                            c = h * 4 + q
                            pst = psum.tile([128, 128], BF16, tag="pt",
                                            bufs=2, padded_shape=[128, 1024])
                            nc.tensor.transpose(
                                pst[:], h1[:, q * 128:(q + 1) * 128],
                                ident_sb[:])
                            nc.vector.tensor_copy(
                                h1Ts[p * 8 + c][:, bt * 128:(bt + 1) * 128],
                                pst[:])

            # ---------- head ----------
            if not do_pe:
                zz = const.tile([1, BC], F32)
                nc.vector.memset(zz[:], 0.0)
                nc.sync.dma_start(out.ap()[0:1, :], zz[:])
                return
            # A = l1w @ [w; b], B = l1w @ [b; w]
            psA = psum.tile([L1, BC], F32, tag="ps")
            for c in range(8):
                nc.tensor.matmul(psA[:], l1w_sb[:, c, :], h1Ts[c][:],
                                 start=(c == 0), stop=False)
            for c in range(8):
                nc.tensor.matmul(psA[:], l1w_sb[:, 8 + c, :], h1Ts[8 + c][:],
                                 start=False, stop=(c == 7))
            psB = psum.tile([L1, BC], F32, tag="ps")
            for c in range(8):
                nc.tensor.matmul(psB[:], l1w_sb[:, c, :], h1Ts[8 + c][:],
                                 start=(c == 0), stop=False)
            for c in range(8):
                nc.tensor.matmul(psB[:], l1w_sb[:, 8 + c, :], h1Ts[c][:],
                                 start=False, stop=(c == 7))

            # h2pre = B + stm*(A-B); then +l1b, clip
            asb = tmppool.tile([L1, BC], F32, tag="asb")
            nc.vector.tensor_copy(asb[:], psA[:])
            d = tmppool.tile([L1, BC], F32, tag="d")
            nc.vector.tensor_sub(d[:], asb[:], psB[:])
            m = tmppool.tile([L1, BC], F32, tag="m")
            nc.vector.tensor_mul(m[:], d[:], stmb_sb[:])
            h2pre = tmppool.tile([L1, BC], F32, tag="h2pre")
            nc.vector.tensor_add(h2pre[:], psB[:], m[:])
            h2f = tmppool.tile([L1, BC], F32, tag="h2f")
            nc.vector.tensor_scalar(h2f[:], h2pre[:], l1b_sb[:], 0.0,
                                    Alu.add, Alu.max)
            h2 = tmppool.tile([L1, BC], BF16, tag="h2")
            nc.vector.tensor_scalar(h2[:], h2f[:], 1.0, None, Alu.min)

            ps2 = psum.tile([L2, BC], F32, tag="ps")
            nc.tensor.matmul(ps2[:], l2w_sb[:], h2[:], start=True, stop=True)
            h3f = tmppool.tile([L2, BC], F32, tag="h3f")
            nc.vector.tensor_scalar(h3f[:], ps2[:], l2b_sb[:], 0.0,
                                    Alu.add, Alu.max)
            h3 = tmppool.tile([L2, BC], BF16, tag="h3")
            nc.vector.tensor_scalar(h3[:], h3f[:], 1.0, None, Alu.min)

            ps3 = psum.tile([1, BC], F32, tag="ps")
            nc.tensor.matmul(ps3[:], l3w_sb[:], h3[:], start=True, stop=True)

            sig_sb = const.tile([1, BC], F32)
            raw_sb = const.tile([1, BC], F32)
            nc.vector.tensor_scalar(raw_sb[:], ps3[:], l3b_sb[:], None,
                                    Alu.add)
            nc.scalar.activation(sig_sb[:], ps3[:], Act.Sigmoid,
                                 bias=l3b_sb[:])
            nc.sync.dma_start(out.ap()[0:1, :], sig_sb[:])
            nc.sync.dma_start(out.ap()[1:2, :], raw_sb[:])

        for _rep in range(reps):
            emit_body()

    nc.compile()
    return nc


def _row_indices(feat_mat):
    """Per-row sorted active-feature indices of a 0/1 matrix."""
    rows, cols = np.nonzero(feat_mat)
    counts = np.bincount(rows, minlength=feat_mat.shape[0])
    split = np.cumsum(counts)[:-1]
    return np.split(cols, split), counts


def _wrap_idx(a):
    """Slot i -> [i % 16, i // 16], replicated to 128 partitions (int16)."""
    m = np.ascontiguousarray(a.reshape(-1, 16).T.astype(np.int16))
    return np.tile(m, (8, 1))


def _wrap_seg(a):
    """Slot i -> [i % 128, i // 128] (f32)."""
    return np.ascontiguousarray(a.reshape(-1, 128).T.astype(np.float32))


def _sort_stream(ii, ss):
    o = np.argsort(ii, kind="stable")
    return ii[o], ss[o]


def _merge_dups(ii, ss):
    """Pair equal adjacent indices (stream sorted by idx) into one slot
    with two segment ids; rows are distinct within a stream (binary
    features), so S entries stay 0/1."""
    n = len(ii)
    if n == 0:
        return ii, ss, ss
    new_group = np.r_[True, ii[1:] != ii[:-1]]
    gid = np.cumsum(new_group) - 1
    first_pos = np.flatnonzero(new_group)
    rank = np.arange(n) - first_pos[gid]
    keep = np.flatnonzero(rank % 2 == 0)
    nxt = keep + 1
    valid = nxt < n
    nxt_c = np.minimum(nxt, n - 1)
    valid &= ii[nxt_c] == ii[keep]
    s2 = np.where(valid, ss[nxt_c], -1)
    return ii[keep], ss[keep], s2


def _prep_sparse(white_features, black_features, stm, ft_w, ft_b,
                 l1_w, l1_b, l2_w, l2_b, l3_w, l3_b):
    """Host prep: bin-pack rows, build tables + index streams.

    Returns (ga, gb, in_maps, perm) or None if the inputs don't fit the
    sparse path.
    """
    import heapq
    f32 = lambda a: np.ascontiguousarray(np.asarray(a, dtype=np.float32))

    white = np.asarray(white_features)
    black = np.asarray(black_features)
    widx, wcnt = _row_indices(white)
    bidx, bcnt = _row_indices(black)

    # bin-pack rows into 32 (core, bt) bins of 128 rows, balancing totals
    tot = wcnt + bcnt
    nbins = NCORES * BT
    order = np.argsort(-tot, kind="stable")
    heap = [(0, b) for b in range(nbins)]
    heapq.heapify(heap)
    bins = [[] for _ in range(nbins)]
    spill = []
    for r in order:
        load, b = heapq.heappop(heap)
        bins[b].append(int(r))
        if len(bins[b]) < 128:
            heapq.heappush(heap, (load + int(tot[r]), b))
        else:
            spill.append((load + int(tot[r]), b))
    assert all(len(rows) == 128 for rows in bins)

    # refine with row swaps: ga/gb (and so all padding and PE work) are set
    # by the max stream length over (bin, persp, half), so hill-climb down
    # the maxes. Per-row lo/hi counts via searchsorted (idx lists sorted).
    wlo_r = np.array([np.searchsorted(widx[r], SPLIT) for r in range(B)])
    blo_r = np.array([np.searchsorted(bidx[r], SPLIT) for r in range(B)])
    whi_r = wcnt - wlo_r
    bhi_r = bcnt - blo_r
    binarr = np.array([bins[b] for b in range(nbins)])  # [nbins, 128]
    lo_bp = np.stack([wlo_r[binarr].sum(1), blo_r[binarr].sum(1)], 1)
    hi_bp = np.stack([whi_r[binarr].sum(1), bhi_r[binarr].sum(1)], 1)

    def cost():
        mlo, mhi = lo_bp.max(), hi_bp.max()
        return ((-(-mlo // 128) + -(-mhi // 128)) * 10**6 + mlo + mhi,)

    rng = np.random.default_rng(0)
    cur = cost()
    for it in range(40000):
        if it % 2 == 0:
            # target a bin holding the current max hi stream
            a = int(np.unravel_index(hi_bp.argmax(), hi_bp.shape)[0])
        else:
            a = int(rng.integers(nbins))
        b2 = int(rng.integers(nbins))
        if a == b2:
            continue
        i = int(rng.integers(128))
        j = int(rng.integers(128))
        r1, r2 = binarr[a, i], binarr[b2, j]
        d_lo = np.array([wlo_r[r2] - wlo_r[r1], blo_r[r2] - blo_r[r1]])
        d_hi = np.array([whi_r[r2] - whi_r[r1], bhi_r[r2] - bhi_r[r1]])
        lo_bp[a] += d_lo
        lo_bp[b2] -= d_lo
        hi_bp[a] += d_hi
        hi_bp[b2] -= d_hi
        new = cost()
        if new <= cur:
            cur = new
            binarr[a, i], binarr[b2, j] = r2, r1
        else:
            lo_bp[a] -= d_lo
            lo_bp[b2] += d_lo
            hi_bp[a] -= d_hi
            hi_bp[b2] += d_hi
    bins = [list(binarr[b]) for b in range(nbins)]

    # build streams per (core, bt, persp, half)
    streams_lo, streams_hi = {}, {}
    max_lo = max_hi = 0
    for b in range(nbins):
        rows = bins[b]
        for p, idx_lists in enumerate((widx, bidx)):
            lo_i, lo_s, hi_i, hi_s = [], [], [], []
            for j, r in enumerate(rows):
                ii = idx_lists[r]
                lo = ii[ii < SPLIT]
                hi = ii[ii >= SPLIT] - SPLIT
                lo_i.append(lo)
                lo_s.append(np.full(len(lo), j))
                hi_i.append(hi)
                hi_s.append(np.full(len(hi), j))
            lo_i = np.concatenate(lo_i) if lo_i else np.empty(0, np.int64)
            lo_s = np.concatenate(lo_s) if lo_s else np.empty(0, np.int64)
            hi_i = np.concatenate(hi_i) if hi_i else np.empty(0, np.int64)
            hi_s = np.concatenate(hi_s) if hi_s else np.empty(0, np.int64)
            # sort each stream by table index (DRAM page locality), then
            # merge duplicate indices: a feature active in two rows of the
            # same stream is gathered once and scattered to both rows via a
            # dual-segment S column (seg2=-1 means no second row)
            lo_i, lo_s, lo_s2 = _merge_dups(*_sort_stream(lo_i, lo_s))
            hi_i, hi_s, hi_s2 = _merge_dups(*_sort_stream(hi_i, hi_s))
            streams_lo[(b, p)] = (lo_i, lo_s, lo_s2)
            streams_hi[(b, p)] = (hi_i, hi_s, hi_s2)
            max_lo = max(max_lo, len(lo_i))
            max_hi = max(max_hi, len(hi_i))

    ga = max(1, math.ceil(max_lo / 128))
    gb = math.ceil(max_hi / 128)
    if ga + gb > GCAP:
        return None

    # pad streams and pack into per-core column-block arrays.
    # Padding slots carry seg=-1 (matches no iota row -> zero S column), so
    # the gathered content for them never reaches the accumulators; idx=0
    # keeps the pad fetches on one DRAM page (row-buffer hits). Every slot
    # is always gathered (static num_idxs): dynamic per-call counts via
    # num_idxs_reg crash the gather ucode (NRT_EXEC_UNIT_UNRECOVERABLE).
    def pack(streams, g):
        cols_i, cols_s, cols_s2 = [], [], []
        for b in range(nbins):
            for p in range(2):
                ii, ss, ss2 = streams[(b, p)]
                n = g * 128
                pi = np.zeros(n, np.int64)
                pi[:len(ii)] = ii
                psg = np.full(n, -1, np.int64)
                psg[:len(ss)] = ss
                psg2 = np.full(n, -1, np.int64)
                psg2[:len(ss2)] = ss2
                cols_i.append(_wrap_idx(pi))
                cols_s.append(_wrap_seg(psg))
                cols_s2.append(_wrap_seg(psg2))
        # group per core: bins are (core*BT + bt)
        per_core_i, per_core_s, per_core_s2 = [], [], []
        for c in range(NCORES):
            blocks = [c * BT * 2 + k for k in range(BT * 2)]
            per_core_i.append(np.concatenate([cols_i[k] for k in blocks],
                                             axis=1))
            per_core_s.append(np.concatenate([cols_s[k] for k in blocks],
                                             axis=1))
            per_core_s2.append(np.concatenate([cols_s2[k] for k in blocks],
                                              axis=1))
        return per_core_i, per_core_s, per_core_s2

    idxlo_c, seglo_c, seg2lo_c = pack(streams_lo, ga)
    if gb:
        idxhi_c, seghi_c, seg2hi_c = pack(streams_hi, gb)

    # tables (fp8 e4m3, scaled by TSCALE into the normal range)
    wT = np.asarray(ft_w, dtype=np.float32).T * TSCALE  # [FEAT, HID]
    tbl = np.zeros((NTA + NTB, HID), F8_NP)
    tbl[0:SPLIT] = wT[0:SPLIT].astype(F8_NP)
    tbl[NTA:NTA + (FEAT - SPLIT)] = wT[SPLIT:FEAT].astype(F8_NP)

    biasb = np.tile((np.asarray(ft_b, np.float32) * TSCALE).astype(BF16_NP),
                    (128, 1))
    ident = np.eye(128, dtype=BF16_NP)
    iota = np.tile(np.arange(128, dtype=np.float32), (128, 1))

    stm_flat = np.asarray(stm, dtype=np.float32).reshape(B)
    bfc = lambda a: np.ascontiguousarray(
        np.asarray(a, dtype=np.float32).astype(BF16_NP))
    l1wT = bfc(np.asarray(l1_w, dtype=np.float32).T / TSCALE)
    l2wT = bfc(np.asarray(l2_w, dtype=np.float32).T)
    l3wT = bfc(np.asarray(l3_w, dtype=np.float32).T)

    perm = np.zeros((NCORES, BC), np.int64)
    in_maps = []
    for c in range(NCORES):
        rows_c = np.concatenate([bins[c * BT + bt] for bt in range(BT)])
        perm[c] = rows_c
        stm_c = np.ascontiguousarray(
            stm_flat[rows_c].reshape(1, BC).astype(BF16_NP))
        m = dict(
            tbl=tbl, idxlo=idxlo_c[c], seglo=seglo_c[c],
            seg2lo=seg2lo_c[c],
            stmv=stm_c, iota=iota, ident=ident, biasb=biasb,
            l1w=l1wT, l1b=f32(l1_b), l2w=l2wT, l2b=f32(l2_b),
            l3w=l3wT, l3b=f32(l3_b))
        if gb:
            m["idxhi"] = idxhi_c[c]
            m["seghi"] = seghi_c[c]
            m["seg2hi"] = seg2hi_c[c]
        in_maps.append(m)
    return ga, gb, in_maps, perm


def _is_binary(x):
    x = np.asarray(x)
    s = x.ravel()[:: max(1, x.size // 65536)]
    if not np.all((s == 0) | (s == 1)):
        return False
    return bool(np.all((x == 0) | (x == 1)))


# --------------------------------------------------------------------------
# Dense fallback path (original kernel)
# --------------------------------------------------------------------------

def _build(feat=FEAT, gk=16, mode="full", reps=1):
    """Build + compile the dense per-core Bass program. Returns nc."""
    kt = feat // 128          # number of K tiles
    assert kt % gk == 0
    ng = kt // gk             # number of K groups
    Alu = mybir.AluOpType
    Act = mybir.ActivationFunctionType

    nc = bacc.Bacc("TRN2", target_bir_lowering=False, debug=False,
                   num_devices=NCORES)

    fw = nc.dram_tensor("fw", (feat, BC), F32, kind="ExternalInput")
    fb = nc.dram_tensor("fb", (feat, BC), F32, kind="ExternalInput")
    wT = nc.dram_tensor("wT", (feat, HID), F32, kind="ExternalInput")
    ftb = nc.dram_tensor("ftb", (HID,), F32, kind="ExternalInput")
    stm = nc.dram_tensor("stm", (BC,), F32, kind="ExternalInput")
    l1w = nc.dram_tensor("l1w", (2 * HID, L1), F32, kind="ExternalInput")
    l1b = nc.dram_tensor("l1b", (L1,), F32, kind="ExternalInput")
    l2w = nc.dram_tensor("l2w", (L1, L2), F32, kind="ExternalInput")
    l2b = nc.dram_tensor("l2b", (L2,), F32, kind="ExternalInput")
    l3w = nc.dram_tensor("l3w", (L2, 1), F32, kind="ExternalInput")
    l3b = nc.dram_tensor("l3b", (1,), F32, kind="ExternalInput")
    out = nc.dram_tensor("out", (2, BC), F32, kind="ExternalOutput")

    with ExitStack() as ctx:
        tc = ctx.enter_context(tile.TileContext(nc))
        const = ctx.enter_context(tc.tile_pool(name="const", bufs=1))
        wpool = ctx.enter_context(tc.tile_pool(name="wpool", bufs=2))
        fwpool = ctx.enter_context(tc.tile_pool(name="fwpool", bufs=2))
        fbpool = ctx.enter_context(tc.tile_pool(name="fbpool", bufs=2))
        accpool = ctx.enter_context(tc.tile_pool(name="accpool", bufs=1))
        h1pool = ctx.enter_context(tc.tile_pool(name="h1pool", bufs=1))
        tmppool = ctx.enter_context(tc.tile_pool(name="tmppool", bufs=1))
        psum = ctx.enter_context(
            tc.tile_pool(name="psum", bufs=8, space="PSUM"))

        # ---------- constants ----------
        ftb_sb = const.tile([128, NHC], F32)
        nc.sync.dma_start(ftb_sb[:], ftb.ap().rearrange("(c p) -> p c", p=128))
        l1b_sb = const.tile([L1, 1], F32)
        nc.sync.dma_start(l1b_sb[:], l1b.ap())
        l2b_sb = const.tile([L2, 1], F32)
        nc.sync.dma_start(l2b_sb[:], l2b.ap())
        l3b_sb = const.tile([1, 1], F32)
        nc.sync.dma_start(l3b_sb[:], l3b.ap())

        l1w_sb = const.tile([128, (2 * HID) // 128, L1], BF16)
        nc.gpsimd.dma_start(l1w_sb[:],
                            l1w.ap().rearrange("(t p) m -> p t m", p=128))
        l2w_sb = const.tile([L1, L2], BF16)
        nc.gpsimd.dma_start(l2w_sb[:], l2w.ap())
        l3w_sb = const.tile([L2, 1], BF16)
        nc.gpsimd.dma_start(l3w_sb[:], l3w.ap())

        stm_bf = const.tile([1, BC], BF16)
        nc.gpsimd.dma_start(stm_bf[:], stm.ap())
        ones_bf = const.tile([1, 128], BF16)
        nc.vector.memset(ones_bf[:], 1.0)

        # broadcast stm across partitions: [128, BC] = ones[1,128].T @ stm[1,BC]
        ps_stm = psum.tile([128, BC], F32, tag="ps")
        nc.tensor.matmul(ps_stm[:], ones_bf[:], stm_bf[:],
                         start=True, stop=True)
        stmb_sb = const.tile([128, BC], F32)
        nc.vector.tensor_copy(stmb_sb[:], ps_stm[:])

        # persistent fp32 accumulators: [0..7] = white persp, [8..15] = black
        accs = [accpool.tile([128, BC], F32, tag=f"acc{i}", name=f"acc{i}")
                for i in range(16)]

        # ---------- feature transformer main loop ----------
        def emit_body():
            sched = [gk] * ng
            roff = 0
            for g, gsz in enumerate(sched):
                r0, r1 = roff * 128, (roff + gsz) * 128
                roff += gsz
                wt = wpool.tile([128, gsz, HID], BF16, tag="wt",
                                name="wt")
                nc.gpsimd.dma_start(
                    wt[:],
                    wT.ap()[r0:r1, :].rearrange("(t p) h -> p t h", p=128))
                fwt = fwpool.tile([128, gsz, BC], BF16, tag="fwt",
                                  name="fwt")
                nc.gpsimd.dma_start(
                    fwt[:],
                    fw.ap()[r0:r1, :].rearrange("(t p) n -> p t n", p=128))
                fbt = fbpool.tile([128, gsz, BC], BF16, tag="fbt",
                                  name="fbt")
                nc.gpsimd.dma_start(
                    fbt[:],
                    fb.ap()[r0:r1, :].rearrange("(t p) n -> p t n", p=128))

                for s, ftile in enumerate((fwt, fbt)):
                    for c in range(NHC):
                        ps = psum.tile([128, BC], F32, tag="ps")
                        for t in range(gsz):
                            nc.tensor.matmul(
                                ps[:],
                                wt[:, t, c * 128:(c + 1) * 128],
                                ftile[:, t, :],
                                start=(t == 0), stop=(t == gsz - 1))
                        a = accs[s * NHC + c]
                        if g == 0:
                            nc.vector.tensor_scalar(
                                a[:], ps[:], ftb_sb[:, c:c + 1], None,
                                Alu.add)
                        else:
                            nc.vector.tensor_add(a[:], a[:], ps[:])

            # ---------- stm select + clip -> h1 (bf16) ----------
            h1s = [h1pool.tile([128, BC], BF16, tag=f"h1_{i}", name=f"h1_{i}")
                   for i in range(16)]
            for c in range(NHC):
                w_, b_ = accs[c], accs[NHC + c]
                d = tmppool.tile([128, BC], F32, tag="d")
                nc.vector.tensor_sub(d[:], w_[:], b_[:])
                m = tmppool.tile([128, BC], F32, tag="m")
                nc.vector.tensor_mul(m[:], d[:], stmb_sb[:])
                topf = tmppool.tile([128, BC], F32, tag="topf")
                nc.vector.tensor_add(topf[:], b_[:], m[:])
                botf = tmppool.tile([128, BC], F32, tag="botf")
                nc.vector.tensor_sub(botf[:], w_[:], m[:])
                nc.gpsimd.tensor_scalar(
                    h1s[c][:], topf[:], 0.0, 1.0, Alu.max, Alu.min)
                nc.gpsimd.tensor_scalar(
                    h1s[NHC + c][:], botf[:], 0.0, 1.0, Alu.max, Alu.min)

            # ---------- head ----------
            ps1 = psum.tile([L1, BC], F32, tag="ps")
            for t in range(16):
                nc.tensor.matmul(ps1[:], l1w_sb[:, t, :], h1s[t][:],
                                 start=(t == 0), stop=(t == 15))
            h2f = tmppool.tile([L1, BC], F32, tag="h2f")
            nc.vector.tensor_scalar(h2f[:], ps1[:], l1b_sb[:], 0.0, Alu.add, Alu.max)
            h2 = tmppool.tile([L1, BC], BF16, tag="h2")
            nc.vector.tensor_scalar(h2[:], h2f[:], 1.0, None, Alu.min)

            ps2 = psum.tile([L2, BC], F32, tag="ps")
            nc.tensor.matmul(ps2[:], l2w_sb[:], h2[:], start=True, stop=True)
            h3f = tmppool.tile([L2, BC], F32, tag="h3f")
            nc.vector.tensor_scalar(h3f[:], ps2[:], l2b_sb[:], 0.0, Alu.add, Alu.max)
            h3 = tmppool.tile([L2, BC], BF16, tag="h3")
            nc.vector.tensor_scalar(h3[:], h3f[:], 1.0, None, Alu.min)

            ps3 = psum.tile([1, BC], F32, tag="ps")
            nc.tensor.matmul(ps3[:], l3w_sb[:], h3[:], start=True, stop=True)

            sig_sb = const.tile([1, BC], F32)
            raw_sb = const.tile([1, BC], F32)
            nc.vector.tensor_scalar(raw_sb[:], ps3[:], l3b_sb[:], None, Alu.add)
            nc.scalar.activation(sig_sb[:], ps3[:], Act.Sigmoid, bias=l3b_sb[:])
            nc.sync.dma_start(out.ap()[0:1, :], sig_sb[:])
            nc.sync.dma_start(out.ap()[1:2, :], raw_sb[:])

        for _rep in range(reps):
            emit_body()

    nc.compile()
    return nc


def _prep_in_maps(white_features, black_features, stm, ft_w, ft_b,
                  l1_w, l1_b, l2_w, l2_b, l3_w, l3_b):
    f32 = lambda a: np.ascontiguousarray(np.asarray(a, dtype=np.float32))
    white = np.asarray(white_features, dtype=np.float32)
    black = np.asarray(black_features, dtype=np.float32)
    stm = np.asarray(stm, dtype=np.float32).reshape(B)
    wT = f32(np.asarray(ft_w, dtype=np.float32).T)        # [FEAT, HID]
    l1wT = f32(np.asarray(l1_w, dtype=np.float32).T)      # [2048, 64]
    l2wT = f32(np.asarray(l2_w, dtype=np.float32).T)      # [64, 32]
    l3wT = f32(np.asarray(l3_w, dtype=np.float32).T)      # [32, 1]
    ftb = f32(ft_b)
    l1b, l2b, l3b = f32(l1_b), f32(l2_b), f32(l3_b)

    in_maps = []
    for c in range(NCORES):
        sl = slice(c * BC, (c + 1) * BC)
        in_maps.append(dict(
            fw=f32(white[sl].T), fb=f32(black[sl].T), wT=wT, ftb=ftb,
            stm=f32(stm[sl]), l1w=l1wT, l1b=l1b, l2w=l2wT, l2b=l2b,
            l3w=l3wT, l3b=l3b))
    return in_maps


# --------------------------------------------------------------------------
# entry point
# --------------------------------------------------------------------------

def kernel(**inputs):
    white = np.asarray(inputs["white_features"])
    black = np.asarray(inputs["black_features"])
    use_sparse = (white.shape == (B, FEAT) and black.shape == (B, FEAT)
                  and _is_binary(white) and _is_binary(black)
                  and _is_binary(inputs["stm"]))
    prep = None
    if use_sparse:
        prep = _prep_sparse(**inputs)
    if prep is not None:
        ga, gb, in_maps, perm = prep
        key = ("sparse", ga, gb)
        if key not in _CACHE:
            _CACHE[key] = _build_sparse(ga, gb)
        _CACHE["last_sparse"] = (ga, gb, in_maps)
        nc = _CACHE[key]
        res = run_bass_kernel_spmd(nc, in_maps, core_ids=list(range(NCORES)))
        sig = np.zeros(B, np.float32)
        raw = np.zeros(B, np.float32)
        for c in range(NCORES):
            sig[perm[c]] = res.results[c]["out"][0]
            raw[perm[c]] = res.results[c]["out"][1]
        return (sig.reshape(B, 1), raw.reshape(B, 1))

    # dense fallback
    if "dense" not in _CACHE:
        _CACHE["dense"] = _build()
    nc = _CACHE["dense"]
    in_maps = _prep_in_maps(**inputs)
    res = run_bass_kernel_spmd(nc, in_maps, core_ids=list(range(NCORES)))
    sig = np.concatenate([res.results[c]["out"][0] for c in range(NCORES)])
    raw = np.concatenate([res.results[c]["out"][1] for c in range(NCORES)])
    return (sig.reshape(B, 1).astype(np.float32),
            raw.reshape(B, 1).astype(np.float32))

